# revision 8
# baseline (speedup 1.0000x reference)
"""Trainium2 Bass kernel for a 2-layer ChebConv (K=5) GNN + global_add_pool + fc.

Strategy (8 NeuronCores, SPMD):
  - dst-shard the edges: core c owns all edges whose dst lands in its node
    shard. Each hop's scatter output is then complete per-core (no
    all-reduce); cores exchange an fp16 node-feature table via AllGather.
  - The per-edge weight (-1/deg[src]) is folded into the published table
    (rows are pre-scaled), so the scatter selection matrix S is 0/1. S is
    hop-invariant and kept SBUF-resident in fp8 (PE matmul takes mixed
    fp16 lhsT x fp8 rhs), eliminating all per-hop S traffic.
  - Gather messages with dma_gather (256B elements = two consecutive fp16
    rows of the node table; per-edge parity selects which half).
  - Scatter-add via PE matmul: out[feat, node_window] += G.T @ S.
  - The node table is split into two halves (by local node id); each half
    has its own AllGather, issued as soon as its blocks are evacuated.
    Next hop consumes half A first, so AG-B overlaps with compute.
  - Chebyshev recurrence fused via PSUM seeding with -Tx0/2 and x2 at
    evacuation.  Transposes + selu interleaved per block (no hop tail).
"""

import os
import sys
import numpy as np

for _p in ("/opt/trn_rl_repo",):
    if os.path.isdir(_p) and _p not in sys.path:
        sys.path.insert(0, _p)

# ---------------------------------------------------------------- config

SELU_L = 1.0507009873554805
SELU_A = 1.6732632423543772


class Cfg:
    def __init__(self, N=100_000, E=1_250_000, NG=64, F=64, K=5, OUT=10,
                 NCORES=8, call_chunks=12):
        self.N, self.E, self.NG, self.F, self.K, self.OUT = N, E, NG, F, K, OUT
        self.NCORES = NCORES
        self.SHARD = (N + NCORES - 1) // NCORES
        self.PSHARD = ((self.SHARD + 127) // 128) * 128
        self.NBLK = (self.PSHARD + 511) // 512   # psum blocks per shard
        self.WIN = 64                            # scatter window (nodes)
        self.CALL_CHUNKS = call_chunks           # chunks per dma_gather call
        # split the node table into buckets by local node id; bucket i covers
        # blocks [bstart[i], bstart[i+1]) of each core's shard
        if self.NBLK >= 2:
            self.BSPLIT = [0, self.NBLK // 2, self.NBLK]
        else:
            self.BSPLIT = [0, self.NBLK]
        self.NBUCKET = len(self.BSPLIT) - 1
        # rows per bucket (padded; last bucket takes the PSHARD remainder)
        self.BROWS = []
        for i in range(self.NBUCKET):
            lo = self.BSPLIT[i] * 512
            hi = min(self.BSPLIT[i + 1] * 512, self.PSHARD)
            self.BROWS.append(hi - lo)


# ---------------------------------------------------------------- host plan


def build_plan(cfg, edge_index):
    """Global (core-independent) chunk structure + per-core S/idx arrays."""
    N, NC = cfg.N, cfg.NCORES
    src = np.asarray(edge_index[0], dtype=np.int64)
    dst = np.asarray(edge_index[1], dtype=np.int64)

    # src table row: bucket by local-node range, row = core*brows + local off
    sl = src % cfg.SHARD
    score = src // cfg.SHARD
    rowsA = cfg.BROWS[0]
    if cfg.NBUCKET == 2:
        bkt_e = (sl >= rowsA).astype(np.int64)
        row = np.where(bkt_e == 0, score * rowsA + sl,
                       score * cfg.BROWS[1] + (sl - rowsA))
    else:
        bkt_e = np.zeros_like(sl)
        row = score * rowsA + sl
    par = (row & 1).astype(np.int64)
    m = row >> 1                               # bucket-local pair index
    for i in range(cfg.NBUCKET):
        assert m[bkt_e == i].max(initial=0) < 32768

    core = dst // cfg.SHARD
    dl = dst % cfg.SHARD                      # dst local node id
    blk = dl // 512

    # group edges by (core, block, bucket, parity), dst-sorted inside
    key = (((core * cfg.NBLK + blk) * cfg.NBUCKET + bkt_e) * 2 + par) * cfg.SHARD + dl
    order = np.argsort(key, kind="stable")
    g_dl, g_m = dl[order], m[order]
    gk = key[order] // cfg.SHARD  # group id per sorted edge
    ngroups = NC * cfg.NBLK * cfg.NBUCKET * 2
    starts = np.searchsorted(gk, np.arange(ngroups + 1))

    def grp(c, b, bk, p):
        gid = ((c * cfg.NBLK + b) * cfg.NBUCKET + bk) * 2 + p
        s, e = starts[gid], starts[gid + 1]
        return g_dl[s:e], g_m[s:e]

    blocks_meta = []   # [b][bkt] -> dict(calls=[(cstart,nch)], wins=[(w0,p)])
    CT = 0
    core_chunks = [[] for _ in range(NC)]

    for b in range(cfg.NBLK):
        bmeta = []
        for bk in range(cfg.NBUCKET):
            data = [[grp(c, b, bk, p) for p in (0, 1)] for c in range(NC)]
            ptr = [[0, 0] for _ in range(NC)]
            wins = []   # (w0, parity) per chunk — single-parity chunks
            run_start = CT
            while True:
                wmin, pmin = None, 0
                for c in range(NC):
                    for p in (0, 1):
                        d = data[c][p][0]
                        if ptr[c][p] < len(d):
                            v = d[ptr[c][p]]
                            if wmin is None or v < wmin:
                                wmin, pmin = v, p
                if wmin is None:
                    break
                wb_psum = min(512, cfg.PSHARD - b * 512)
                w0 = min(int(wmin) - b * 512,
                         max(0, wb_psum - cfg.WIN))
                limit = b * 512 + w0 + cfg.WIN
                cid = CT
                p = pmin
                for c in range(NC):
                    d, mm = data[c][p]
                    lo = ptr[c][p]
                    hi = np.searchsorted(d, limit, side="left")
                    take = min(128, hi - lo)
                    if take > 0:
                        core_chunks[c].append(
                            (cid, d[lo:lo + take] - b * 512 - w0,
                             mm[lo:lo + take]))
                        ptr[c][p] = lo + take
                wins.append((w0, p))
                CT += 1
            nch_run = CT - run_start
            calls = []
            off = 0
            while off < nch_run:
                n = min(cfg.CALL_CHUNKS, nch_run - off)
                calls.append((run_start + off, n))
                off += n
            bmeta.append({"calls": calls, "wins": wins, "cstart": run_start,
                          "nch": nch_run})
        blocks_meta.append(bmeta)

    # materialize per-core arrays
    import ml_dtypes
    S_list, idx_list = [], []
    for c in range(NC):
        S = np.zeros((CT, 128, cfg.WIN), dtype=ml_dtypes.float8_e4m3)
        idx = np.zeros((CT, 128), dtype=np.int16)
        for cid, dls, ms in core_chunks[c]:
            n = len(dls)
            rows = np.arange(n)
            S[cid, rows, dls] = 1.0
            idx[cid, rows] = ms.astype(np.int16)
        # wrap idx per call: position i -> [i%16, i//16]
        idx_w = np.zeros((16, CT * 8), dtype=np.int16)
        for bmeta in blocks_meta:
            for bm in bmeta:
                for (cs, n) in bm["calls"]:
                    flat = idx[cs:cs + n].reshape(-1)           # [n*128]
                    idx_w[:, cs * 8:(cs + n) * 8] = flat.reshape(-1, 16).T
        S_flat = np.ascontiguousarray(
            S.transpose(1, 0, 2).reshape(128, CT * cfg.WIN))
        S_list.append(S_flat)
        idx_list.append(np.ascontiguousarray(np.tile(idx_w, (8, 1))))

    deg = np.bincount(src, minlength=N).astype(np.float64)
    ninvdeg = np.where(deg > 0, -1.0 / np.maximum(deg, 1), 0.0).astype(np.float32)
    return {"CT": CT, "blocks": blocks_meta, "S": S_list, "idx": idx_list,
            "ninvdeg": ninvdeg}


def build_host_inputs(cfg, plan, x, batch, W1, b1, W2, b2, Wfc, bfc):
    """Per-core in_map dicts."""
    import ml_dtypes
    N, F, NG = cfg.N, cfg.F, cfg.NG
    x = np.asarray(x, np.float32)
    batch = np.asarray(batch, np.int64)
    ninv = plan["ninvdeg"]
    xs = x * ninv[:, None]                      # pre-scaled table rows
    W_sb = np.zeros((128, 2 * cfg.K * F), np.float16)
    for l, W in enumerate((W1, W2)):
        for k in range(cfg.K):
            blkc = (l * cfg.K + k) * F
            r0 = (k % 2) * 64          # W_k contracts state rows of Tx_k
            W_sb[r0:r0 + 64, blkc:blkc + F] = W[k]
    b12 = np.stack([np.asarray(b1, np.float32), np.asarray(b2, np.float32)], axis=1)
    ident = np.zeros((128, 64), np.float16)
    ident[np.arange(128), np.arange(128) % 64] = 1.0
    neghalf = np.zeros((128, 128), np.float16)
    neghalf[np.arange(128), np.arange(128)] = -0.5
    ngrp = cfg.PSHARD // 128

    # bucket tables (replicated across cores)
    t0 = []
    for i in range(cfg.NBUCKET):
        rows = cfg.BROWS[i]
        lo = cfg.BSPLIT[i] * 512
        t = np.zeros((cfg.NCORES * rows, F), np.float16)
        for c in range(cfg.NCORES):
            nlo, nhi = c * cfg.SHARD + lo, min(c * cfg.SHARD + lo + rows, N)
            nhi = max(nhi, nlo)
            ns = min(nhi - nlo, cfg.SHARD - lo) if lo < cfg.SHARD else 0
            ns = max(ns, 0)
            if ns > 0:
                t[c * rows:c * rows + ns] = xs[nlo:nlo + ns].astype(np.float16)
        t0.append(t)

    in_maps = []
    for c in range(cfg.NCORES):
        lo, hi = c * cfg.SHARD, min((c + 1) * cfg.SHARD, N)
        ns = hi - lo
        x_fm = np.zeros((64, cfg.PSHARD), np.float16)
        x_fm[:, :ns] = x[lo:hi].T.astype(np.float16)
        bt = np.zeros((128, ngrp * NG), ml_dtypes.float8_e4m3)
        l_ = np.arange(ns)
        bt[l_ % 128, (l_ // 128) * NG + batch[lo:hi]] = 1.0
        ninv_c = np.zeros((128, ngrp), np.float32)
        ninv_c[l_ % 128, l_ // 128] = ninv[lo:hi]
        im = {
            "x_fm": x_fm,
            "s8_in": plan["S"][c],
            "idx_all": plan["idx"][c],
            "bt_in": bt,
            "w_sb_in": W_sb,
            "b12_in": b12,
            "wfc_in": np.asarray(Wfc, np.float32),
            "bfc_in": np.asarray(bfc, np.float32).reshape(cfg.OUT, 1),
            "ident_in": ident,
            "neghalf_in": neghalf,
            "ninv_in": ninv_c,
        }
        for i in range(cfg.NBUCKET):
            im[f"table0_{i}"] = t0[i]
        in_maps.append(im)
    return in_maps


# ---------------------------------------------------------------- device


def build_kernel(cfg, plan, nprop=None):
    import concourse.bass as bass
    import concourse.bacc as bacc
    import concourse.mybir as mybir
    import concourse.tile as tile

    dt = mybir.dt
    F, K, NG, OUT = cfg.F, cfg.K, cfg.NG, cfg.OUT
    PSH, CT, WIN = cfg.PSHARD, plan["CT"], cfg.WIN
    NBLK, NBUCKET = cfg.NBLK, cfg.NBUCKET
    ngrp = PSH // 128
    NC = cfg.NCORES
    # groups per bucket
    GSPLIT = [min(s * 4, ngrp) for s in cfg.BSPLIT]

    nc = bacc.Bacc("TRN2", debug=False, target_bir_lowering=False,
                   num_devices=NC,
                   dynamic_dma_scratch_size=49152)

    # I/O
    x_fm_t = nc.dram_tensor("x_fm", [64, PSH], dt.float16, kind="ExternalInput")
    t0_t = [nc.dram_tensor(f"table0_{i}", [NC * cfg.BROWS[i], F], dt.float16,
                           kind="ExternalInput") for i in range(NBUCKET)]
    s8_t = nc.dram_tensor("s8_in", [128, CT * WIN], dt.float8e4, kind="ExternalInput")
    idx_all_t = nc.dram_tensor("idx_all", [128, CT * 8], dt.int16, kind="ExternalInput")
    bt_t = nc.dram_tensor("bt_in", [128, ngrp * NG], dt.float8e4, kind="ExternalInput")
    w_sb_t = nc.dram_tensor("w_sb_in", [128, 2 * K * F], dt.float16, kind="ExternalInput")
    b12_t = nc.dram_tensor("b12_in", [64, 2], dt.float32, kind="ExternalInput")
    wfc_t = nc.dram_tensor("wfc_in", [64, OUT], dt.float32, kind="ExternalInput")
    bfc_t = nc.dram_tensor("bfc_in", [OUT, 1], dt.float32, kind="ExternalInput")
    ident_t = nc.dram_tensor("ident_in", [128, 64], dt.float16, kind="ExternalInput")
    neghalf_t = nc.dram_tensor("neghalf_in", [128, 128], dt.float16, kind="ExternalInput")
    ninv_t = nc.dram_tensor("ninv_in", [128, ngrp], dt.float32, kind="ExternalInput")
    out_t = nc.dram_tensor("out_t", [OUT, NG], dt.float32, kind="ExternalOutput")

    rg = [list(range(NC))]
    skip_gather = bool(int(os.environ.get("KSKIP_GATHER", "0")))
    skip_ag = bool(int(os.environ.get("KSKIP_AG", "0")))
    skip_trans = bool(int(os.environ.get("KSKIP_TRANS", "0")))

    with tile.TileContext(nc) as tc:
        with (
            tc.tile_pool(name="const", bufs=1) as cpool,
            tc.tile_pool(name="state", bufs=1) as spool,
            tc.tile_pool(name="gather", bufs=3) as gpool,
            tc.tile_pool(name="idx", bufs=3) as ipool,
            tc.tile_pool(name="psum_y", bufs=2, space="PSUM") as pyp,
            tc.tile_pool(name="psum_w", bufs=2, space="PSUM") as pwp,
            tc.tile_pool(name="psum_t", bufs=2, space="PSUM") as ptp,
            tc.tile_pool(name="dram", bufs=1, space="DRAM") as dpool,
        ):
            # ---- constants to SBUF
            w_sb = cpool.tile([128, 2 * K * F], dt.float16)
            b12_sb = cpool.tile([64, 2], dt.float32)
            wfc_sb = cpool.tile([64, OUT], dt.float32)
            bfc_sb = cpool.tile([OUT, 1], dt.float32)
            ident_sb = cpool.tile([128, 64], dt.float16)
            neghalf_sb = cpool.tile([128, 128], dt.float16)
            ninv_sb = cpool.tile([128, ngrp], dt.float32)
            bt_sb = cpool.tile([128, ngrp * NG], dt.float8e4)
            s8_sb = cpool.tile([128, CT * WIN], dt.float8e4)
            nc.sync.dma_start(out=w_sb[:], in_=w_sb_t[:])
            nc.sync.dma_start(out=b12_sb[:], in_=b12_t[:])
            nc.sync.dma_start(out=wfc_sb[:], in_=wfc_t[:])
            nc.sync.dma_start(out=bfc_sb[:], in_=bfc_t[:])
            nc.sync.dma_start(out=ident_sb[:], in_=ident_t[:])
            nc.sync.dma_start(out=neghalf_sb[:], in_=neghalf_t[:])
            nc.sync.dma_start(out=ninv_sb[:], in_=ninv_t[:])
            nc.sync.dma_start(out=bt_sb[:], in_=bt_t[:])
            nc.sync.dma_start(out=s8_sb[:], in_=s8_t[:])

            # ---- state
            stA = spool.tile([128, PSH], dt.float16)   # halves: Tx even / odd
            out_sb = spool.tile([64, PSH], dt.float16)
            nm_sb = [spool.tile([128, (GSPLIT[i + 1] - GSPLIT[i]) * F],
                                dt.float16, name=f"nm{i}")
                     for i in range(NBUCKET)]
            p_sb = spool.tile([64, 512], dt.float16)   # selu pos part scratch
            g_sb = spool.tile([64, NG], dt.float32)
            gfull_sb = spool.tile([64, NG], dt.float32)
            o_sb = spool.tile([OUT, NG], dt.float32)

            nc.sync.dma_start(out=stA[0:64, :], in_=x_fm_t[:])

            # ---- DRAM
            NPROP = 2 * (K - 1) if nprop is None else nprop
            stage = [dpool.tile([cfg.BROWS[i], F], dt.float16, name=f"stage{i}")
                     for i in range(NBUCKET)]
            # one AllGather output tile per (bucket, hop) — Shared tiles
            # require a single writing instruction
            tbuf = [[dpool.tile([NC * cfg.BROWS[i], F], dt.float16,
                                name=f"tbuf{i}_{j}", addr_space="Shared")
                     for j in range(max(NPROP - 1, 0))] for i in range(NBUCKET)]
            gt_in = dpool.tile([64, NG], dt.float32)
            gt_out = dpool.tile([64, NG], dt.float32, addr_space="Shared")

            def gather_src(h, bk):
                t = t0_t[bk] if h == 0 else tbuf[bk][h - 1]
                return t[:].rearrange("(r two) f -> r (two f)", two=2)

            for h in range(NPROP):
                l, k = h // (K - 1), h % (K - 1) + 1
                hc = (k % 2) * 64          # partition base of Tx_k
                last_hop = (h == 2 * (K - 1) - 1)
                do_table = not skip_trans

                for b in range(NBLK):
                    w_b = min(512, PSH - b * 512)
                    bc = slice(b * 512, b * 512 + w_b)
                    # last (bucket, chunk) of this block, for PSUM stop
                    bk_last, cid_last = -1, -1
                    if not skip_gather:
                        for bk in range(NBUCKET - 1, -1, -1):
                            bm = plan["blocks"][b][bk]
                            if bm["nch"] > 0:
                                bk_last = bk
                                cid_last = bm["cstart"] + bm["nch"] - 1
                                break
                    psum_y = pyp.tile([128, 512], dt.float32)
                    if k == 1:
                        nc.vector.memset(psum_y[hc:hc + 64, :w_b], 0.0)
                    else:
                        # psum := -Tx_{k-2}/2
                        nc.tensor.matmul(
                            psum_y[hc:hc + 64, :w_b],
                            neghalf_sb[:, hc:hc + 64],
                            stA[:, bc],
                            start=True, stop=(bk_last < 0),
                            skip_group_check=True)
                    for bk in (range(NBUCKET) if not skip_gather else []):
                        bm = plan["blocks"][b][bk]
                        src_ap = gather_src(h, bk)
                        for (cs, nch) in bm["calls"]:
                            it = ipool.tile([128, cfg.CALL_CHUNKS * 8], dt.int16)
                            gt = gpool.tile([128, cfg.CALL_CHUNKS * 128], dt.float16)
                            nc.sync.dma_start(out=it[:, :nch * 8],
                                              in_=idx_all_t[:, cs * 8:(cs + nch) * 8])
                            L = nch * 128
                            nc.gpsimd.dma_gather(
                                gt[:, :L].rearrange("p (c f) -> p c f", f=128),
                                src_ap, it[:, :nch * 8], L, L, 128,
                                single_packet=False)
                            for j in range(nch):
                                cid = cs + j
                                w, par = bm["wins"][cid - bm["cstart"]]
                                c0 = j * 128 + par * 64
                                nc.tensor.matmul(
                                    psum_y[hc:hc + 64, w:w + WIN],
                                    gt[:, c0:c0 + 64],
                                    s8_sb[:, cid * WIN:(cid + 1) * WIN],
                                    start=False,
                                    stop=(bk == bk_last and cid == cid_last),
                                    skip_group_check=True)
                    # evacuate: Tx_k
                    if k == 1:
                        nc.vector.tensor_copy(stA[hc:hc + 64, bc],
                                              psum_y[hc:hc + 64, :w_b])
                    else:
                        nc.vector.tensor_scalar(stA[hc:hc + 64, bc],
                                                psum_y[hc:hc + 64, :w_b],
                                                2.0, None, mybir.AluOpType.mult)
                    # out += Tx_k @ W_k  (transposed: psum_w = W_k.T @ Tx_k)
                    psum_w = pwp.tile([64, 512], dt.float32)
                    wc = (l * K + k) * F
                    nc.tensor.matmul(psum_w[:, :w_b],
                                     w_sb[:, wc:wc + F],
                                     stA[:, bc],
                                     start=True, stop=(k != 1),
                                     skip_group_check=True)
                    if k == 1:  # also Tx0 @ W0
                        nc.tensor.matmul(psum_w[:, :w_b],
                                         w_sb[:, (l * K) * F:(l * K) * F + F],
                                         stA[:, bc],
                                         start=False, stop=True,
                                         skip_group_check=True)
                        nc.vector.tensor_copy(out_sb[:, bc], psum_w[:, :w_b])
                    else:
                        nc.vector.tensor_tensor(out_sb[:, bc], out_sb[:, bc],
                                                psum_w[:, :w_b],
                                                mybir.AluOpType.add)

                    hsrc = hc
                    if k == K - 1:
                        # ---- h = selu(out + b_l) -> stA[0:64, bc]
                        lam, alpha = SELU_L, SELU_A
                        nc.vector.tensor_scalar(out_sb[:, bc], out_sb[:, bc],
                                                b12_sb[:, l:l + 1], None,
                                                mybir.AluOpType.add)
                        nc.scalar.activation(p_sb[:, :w_b], out_sb[:, bc],
                                             mybir.ActivationFunctionType.Relu,
                                             scale=lam)
                        nc.vector.tensor_scalar(out_sb[:, bc], out_sb[:, bc],
                                                0.0, None, mybir.AluOpType.min)
                        nc.scalar.activation(out_sb[:, bc], out_sb[:, bc],
                                             mybir.ActivationFunctionType.Exp)
                        nc.vector.tensor_scalar(out_sb[:, bc], out_sb[:, bc],
                                                lam * alpha, -lam * alpha,
                                                mybir.AluOpType.mult,
                                                mybir.AluOpType.add)
                        nc.vector.tensor_tensor(stA[0:64, bc], out_sb[:, bc],
                                                p_sb[:, :w_b],
                                                mybir.AluOpType.add)
                        hsrc = 0

                    # ---- node-major fp16 table rows for this block
                    if do_table:
                        for g in range(4 * b, min(4 * b + 4, ngrp)):
                            bi = 0
                            while g >= GSPLIT[bi + 1]:
                                bi += 1
                            gl = g - GSPLIT[bi]
                            pt = ptp.tile([128, 64], dt.float16)
                            nc.tensor.matmul(
                                pt[:],
                                stA[hsrc:hsrc + 64, g * 128:(g + 1) * 128],
                                ident_sb[hsrc:hsrc + 64, :],
                                is_transpose=True, skip_group_check=True)
                            if last_hop:
                                nc.vector.tensor_copy(
                                    nm_sb[bi][:, gl * F:(gl + 1) * F], pt[:])
                            else:
                                nc.vector.tensor_scalar(
                                    nm_sb[bi][:, gl * F:(gl + 1) * F], pt[:],
                                    ninv_sb[:, g:g + 1], None,
                                    mybir.AluOpType.mult)
                    # ---- ship buckets whose blocks are now all done
                    if do_table and not last_hop and not skip_ag:
                        for bi in range(NBUCKET):
                            if b == cfg.BSPLIT[bi + 1] - 1:
                                nc.sync.dma_start(
                                    out=stage[bi][:].rearrange(
                                        "(g p) f -> p g f", p=128),
                                    in_=nm_sb[bi][:].rearrange(
                                        "p (g f) -> p g f", f=F))
                                nc.gpsimd.collective_compute(
                                    "AllGather", mybir.AluOpType.bypass,
                                    replica_groups=rg,
                                    ins=[stage[bi].opt()],
                                    outs=[tbuf[bi][h].opt()])

            # ---- pooling: gT = sum_n h2[n] per graph  (psum[64f, NG])
            if not skip_trans:
                psum_g = pwp.tile([64, 512], dt.float32, tag="psum_w")
                for g in range(ngrp):
                    bi = 0
                    while g >= GSPLIT[bi + 1]:
                        bi += 1
                    gl = g - GSPLIT[bi]
                    nc.tensor.matmul(psum_g[:, :NG],
                                     nm_sb[bi][:, gl * F:(gl + 1) * F],
                                     bt_sb[:, g * NG:(g + 1) * NG],
                                     start=(g == 0), stop=(g == ngrp - 1),
                                     skip_group_check=True)
                nc.vector.tensor_copy(g_sb[:], psum_g[:, :NG])
            else:
                nc.vector.memset(g_sb[:], 0.0)
            nc.sync.dma_start(out=gt_in[:], in_=g_sb[:])
            nc.gpsimd.collective_compute(
                "AllReduce", mybir.AluOpType.add, replica_groups=rg,
                ins=[gt_in.opt()], outs=[gt_out.opt()])
            nc.sync.dma_start(out=gfull_sb[:], in_=gt_out[:])
            psum_o = ptp.tile([128, 64], dt.float32, tag="pt")
            nc.tensor.matmul(psum_o[0:OUT, 0:NG],
                             wfc_sb[:],
                             gfull_sb[:],
                             start=True, stop=True, skip_group_check=True)
            nc.vector.tensor_scalar(o_sb[:], psum_o[0:OUT, 0:NG],
                                    bfc_sb[:, 0:1], None, mybir.AluOpType.add)
            nc.sync.dma_start(out=out_t[:], in_=o_sb[:])

    nc.compile()
    return nc


# ---------------------------------------------------------------- entry


def run(cfg, inputs, trace=False):
    from concourse.bass_utils import run_bass_kernel_spmd
    edge_index = np.asarray(inputs["edge_index"])
    plan = build_plan(cfg, edge_index)
    nprop = int(os.environ.get("KNPROP", "0")) or None
    nc = build_kernel(cfg, plan, nprop=nprop)
    in_maps = build_host_inputs(
        cfg, plan, inputs["x"], inputs["batch"],
        inputs["W1"], inputs["b1"], inputs["W2"], inputs["b2"],
        inputs["Wfc"], inputs["bfc"])
    core_ids = list(range(cfg.NCORES))
    res = run_bass_kernel_spmd(nc, in_maps, core_ids, trace=trace)
    out = np.asarray(res.results[0]["out_t"]).T.copy()  # [NG, OUT]
    return out, res


def kernel(**inputs):
    cfg = Cfg()
    out, _ = run(cfg, inputs, trace=False)
    return out.astype(np.float32)
